# revision 26
# baseline (speedup 1.0000x reference)
"""DebiasedPosLossV2 contrastive loss on 8 Trainium2 NeuronCores.

Math (reference, B=4096, D=128, TEMP=0.5, TAU=0.1):
    out = concat([out_1, out_2])            # [2B, D], rows L2-normalized
    sim = exp(out @ out.T / TEMP)           # [2B, 2B]
    full_i = sum_j sim_ij
    keep_ij = (j%B != i%B) & ~(t_i == t_j)  where t = concat([target, target])
    Ng_i = sum_j keep_ij * sim_ij
    loss = mean(-log(o1/o2)),  o1 = full - .9*Ng,  o2 = full + (n*.1-.9)*Ng

Key identity: t_j == t_i whenever j%B == i%B, so keep_ij == (t_i != t_j) and
    Ng_i = full_i - S_i,   S_i = sum_{j: t_j == t_i} sim_ij.

Sharding: every core holds the full X^T (all-gather done host-side by
replication) and owns a 1024-column strip of sim; column sums equal row sums
by symmetry. Per 512-column chunk, the core accumulates over all 64 row
blocks rb:
    Q[c, j] = sum_i [t_i == c] ez[i, j]   (one-hot matmul; row 0 = ones
                                           column -> full_j)
then extracts full_j = Q[0, j] and S_j = Q[1 + t_j, j] on-device (cmask
multiply + ones-matmul partition reduce) and ships only [full | S] = 4KB per
chunk; a [101, 512] Q dump was measured at 21 GB/s on a single DMA engine
(9.7us of tail), so the output must stay tiny. Host finishes with
o1/o2/log/mean in float64.

Engine budget per core: exp() over 8.4M elements would be 54.6us on ScalarE
alone (1 elem/cycle/lane @1.2GHz), and the two matmuls per 128x512 block
put TensorE at 55us (1 column/cycle @2.4GHz). Three measures get under
both limits:
  - exp is split across engines: even groups get true ScalarE exp into
    fp8e4m3; odd groups get a Schraudolph bit-trick exp on the otherwise
    idle VectorE: bits = int8(SCH8_A*z + SCH8_B) viewed as fp8, which is
    exp(2z) with ~+-5% ripple. The ripple is value-dependent but
    target-independent, so it applies the same multiplicative factor (in
    expectation) to full_j and S_j; o1/o2 are linear in (full, S), so the
    common factor cancels in o2/o1 and the final loss error is ~1e-5,
    far inside the 2e-2 budget.
  - everything is fp8e4m3: x entries (|x|<=1, unit rows) quantize to
    ~1.8% rel err, zero-mean, which washes out over 8192-term sums; input
    DMA bytes halve vs fp16.
  - the one-hot reduce runs as ONE DoubleRow fp8 matmul per row-block
    PAIR (stationary [128, 2, 128], moving [128, 2, 512], K=256): half
    the reduce instructions at 0.5 cycles/row, cutting TensorE to ~43us,
    which is the critical path.

Startup: DMA issues are spread across the sync/scalar/gpsimd queues (each
issue costs ~600ns serially per engine; walrus caps instructions at one
sync wait, so extra data/recycle waits are hoisted onto same-engine DRAIN
chains), and ~30 dummy N=128 matmuls on a memset tile warm the PE's HAM
clock gate (cold PE runs at 1.2GHz for the first ~3.4us of activity) while
the boot DMA is still in flight.
"""

import sys

if "/opt/trn_rl_repo" not in sys.path:
    sys.path.insert(0, "/opt/trn_rl_repo")

from contextlib import ExitStack

import numpy as np

import concourse.bass as bass
import concourse.mybir as mybir
import concourse.tile as tile
from concourse.bass import ds, ts
from concourse.bass_utils import run_bass_kernel_spmd

B = 4096
D = 128
TWO_B = 2 * B
TEMPERATURE = 0.5
TAU_PLUS = 0.1
N_CORES = 8
COLS_PER_CORE = TWO_B // N_CORES  # 1024
CHUNK = 512                       # psum bank width (fp32)
N_CHUNKS = COLS_PER_CORE // CHUNK  # 2
N_RB = TWO_B // 128               # 64 row blocks
G = 2                             # row blocks per group (z tile = 2 banks)
N_GROUPS = N_RB // G              # 32 groups per chunk
NCLS = 100                        # target values in [0, 100)
# one-hot layout: col 0 = ones (-> full row of Q), cols 1..100 = classes,
# cols 101..127 = zero pad (keeps PSUM APs partition-0 based and the
# 128-wide weight tile enables fast weight load).
OHW = 128
NWARM = 26                        # HAM warm-up matmuls (N=128 each)

# Schraudolph fp16 exp(2z): bits = int16(SCH_A*z + SCH_B) viewed as fp16.
# SCH_A = 1024 * 2/ln2; SCH_B = 15*1024 - 44.07 (centers the +-3% piecewise-
# linear ripple) + 0.5 (int conversion truncates).
SCH_A = 2954.639443
SCH_B = 15316.43
# Schraudolph fp8e4m3 exp(2z): bits = int8(SCH8_A*z + SCH8_B) viewed as fp8.
SCH8_A = 23.083120
SCH8_B = 56.1557

F16 = mybir.dt.float16
F32 = mybir.dt.float32
I16 = mybir.dt.int16
I8 = mybir.dt.int8
F8 = mybir.dt.float8e4

_PROGRAM = None
_PROGRAM_SPLIT = False


def group_on_dve(c: int, g: int) -> bool:
    """Which engine exponentiates group g of chunk c: False=ScalarE (true
    exp), True=VectorE (Schraudolph). Alternate for balanced load; chunk 1's
    group 5 goes to ScalarE so the DVE can absorb chunk-0's extract ops
    (mask-mul + stile copy, ~1.4us) without backing up the exp pipeline."""
    if c == 1 and g == 5:
        return False
    return g % 2 == 1


def _build_program() -> bass.Bass:
    nc = bass.Bass()

    # boot: everything group 0 needs in ONE descriptor:
    # [xt cols 0:256 | xtc chunk 0 | oh blocks 0,1] fp16 [128, 1024]
    boot_d = nc.declare_dram_parameter("boot", [128, 2 * 128 + CHUNK], F8, isOutput=False)
    # w0 = xt cols 256:1024
    w0_d = nc.declare_dram_parameter("w0", [128, 768], F8, isOutput=False)
    # wk[k] = xt cols (k+1)*1024:(k+2)*1024
    wk_d = nc.declare_dram_parameter("wk", [7, 128, 1024], F8, isOutput=False)
    # one-hot pair weights for the DoubleRow reduce: [p, pair, k, class]
    oh8a_d = nc.declare_dram_parameter("oh8a", [128, 4, 2, OHW], F8, isOutput=False)
    oh8b_d = nc.declare_dram_parameter("oh8b", [128, 12, 2, OHW], F8, isOutput=False)
    oh8c_d = nc.declare_dram_parameter("oh8c", [128, 16, 2, OHW], F8, isOutput=False)
    xtc1_d = nc.declare_dram_parameter("xtc1", [D, CHUNK], F8, isOutput=False)
    cm_d = nc.declare_dram_parameter("cmask", [NCLS + 1, COLS_PER_CORE], F8, isOutput=False)
    sel_d = nc.declare_dram_parameter("sel", [NCLS + 1, 2], F16, isOutput=False)
    fs_d = [
        nc.declare_dram_parameter(f"fs{c}", [2, CHUNK], F32, isOutput=True)
        for c in range(N_CHUNKS)
    ]

    with ExitStack() as ctx:
        tc = ctx.enter_context(tile.TileContext(nc))
        const = ctx.enter_context(tc.tile_pool(name="const", bufs=1))
        ezp = ctx.enter_context(tc.tile_pool(name="ez", bufs=4))
        mkp = ctx.enter_context(tc.tile_pool(name="mk", bufs=2))
        fsp = ctx.enter_context(tc.tile_pool(name="fs", bufs=2))
        zp = ctx.enter_context(tc.tile_pool(name="z", bufs=3, space="PSUM"))
        qp = ctx.enter_context(tc.tile_pool(name="q", bufs=2, space="PSUM"))

        # --- SBUF tiles ---
        warm = const.tile([128, 128], F8, tag="warm")
        boot = const.tile([128, 2 * 128 + CHUNK], F8, tag="boot")
        w0 = const.tile([128, 768], F8, tag="w0")
        wks = [
            const.tile([128, 1024], F8, tag=f"wk{k}", name=f"wk{k}")
            for k in range(1, 8)
        ]
        ohsb = const.tile([128, N_GROUPS, 2, OHW], F8, tag="ohsb")
        xtc1 = const.tile([D, CHUNK], F8, tag="xtc1")

        # --- DMA issue schedule: spread across queues; each issue costs
        # ~600ns serially on its engine, and the boot transfer gates the
        # first real matmul, so boot goes first on sync while gpsimd memsets
        # the warm-up tile and vector fetches w0 in parallel. ---
        cm = const.tile([NCLS + 1, COLS_PER_CORE], F8, tag="cm")
        sel = const.tile([NCLS + 1, 2], F16, tag="sel")
        nc.gpsimd.memset(warm[:], 1.0)
        # gpsimd's SWDGE moves big transfers through one engine at ~26GB/s
        # (a 384KB input there stalled the PE 13us) -- inputs go ONLY on the
        # two hardware queues. sync's SP engine is otherwise idle, so it
        # carries the long list; scalar keeps <=3 so the auto-inserted
        # ACT_TABLE_LOAD (and first exp) isn't pushed past ~11us.
        nc.sync.dma_start(boot[:], boot_d[:])
        nc.scalar.dma_start(w0[:], w0_d[:])        # own queue: lands ~ with boot
        nc.sync.dma_start(ohsb[:, 0:4], oh8a_d[:])
        nc.scalar.dma_start(wks[0][:], wk_d[0])    # rb 8-15, needed ~13us
        nc.scalar.dma_start(ohsb[:, 4:16], oh8b_d[:])
        nc.scalar.dma_start(sel[:], sel_d[:])
        nc.sync.dma_start(wks[1][:], wk_d[1])
        nc.sync.dma_start(wks[2][:], wk_d[2])
        nc.sync.dma_start(wks[3][:], wk_d[3])
        nc.sync.dma_start(wks[4][:], wk_d[4])
        nc.sync.dma_start(wks[5][:], wk_d[5])
        nc.sync.dma_start(wks[6][:], wk_d[6])
        nc.sync.dma_start(ohsb[:, 16:32], oh8c_d[:])
        nc.sync.dma_start(cm[:], cm_d[:])
        nc.sync.dma_start(xtc1[:], xtc1_d[:])      # chunk 1, needed ~40us

        xtc_h = [boot[:, 256 : 256 + CHUNK], xtc1[:]]

        def w1(rb):  # lhsT for the z matmul of row block rb
            if rb < 2:
                return boot[:, ts(rb, 128)]
            if rb < 8:
                return w0[:, ts(rb - 2, 128)]
            return wks[rb // 8 - 1][:, ts(rb % 8, 128)]

        def w2pair(p):  # [128, 2, OHW] stationary pair for the DR reduce
            return ohsb[:, p]

        # --- PE HAM warm-up: ~30 junk matmuls (N=128) on the memset tile.
        # They burn the ~3.4us activity window while the boot DMA is in
        # flight so the real matmul stream starts at 2.4GHz. Output goes to
        # a z-pool tile that is recycled before the real groups need it. ---
        zw = zp.tile([128, G * CHUNK], F32, tag="z", name="zwarm")
        for _ in range(NWARM):
            nc.tensor.matmul(
                zw[:, 0:128],
                lhsT=warm[:],
                rhs=warm[:],
                start=True,
                stop=True,
                skip_group_check=True,
            )

        def emit_exp(z_slice, ez_ap, on_dve):
            """One exp pass over a z PSUM slice into an fp8 (or int8) SBUF
            tile AP; returns the fp8-typed AP the reduce matmul streams."""
            if on_dve:
                nc.vector.tensor_scalar(
                    ez_ap,
                    z_slice,
                    SCH8_A,
                    SCH8_B,
                    op0=mybir.AluOpType.mult,
                    op1=mybir.AluOpType.add,
                )
                return ez_ap.bitcast(F8)
            nc.scalar.activation(
                ez_ap,
                z_slice,
                mybir.ActivationFunctionType.Exp,
                scale=1.0 / TEMPERATURE,
            )
            return ez_ap

        NSPLIT = 2

        def emit_split_group(c, q, g):
            """One group as two G=1 halves, ScalarE + VectorE in parallel."""
            rbs = [G * g, G * g + 1]
            z = zp.tile([128, G * CHUNK], F32, tag="z", name="z")
            for s, rb in enumerate(rbs):
                nc.tensor.matmul(
                    z[:, ts(s, CHUNK)],
                    lhsT=w1(rb),
                    rhs=xtc_h[c],
                    start=True,
                    stop=True,
                    skip_group_check=True,
                )
            # dedicated tiles: pool reuse would add cross-engine WAW
            # waits; walrus caps compute instructions at one sync wait
            eza = const.tile([128, CHUNK], F8, tag=f"ezta{c}_{g}")
            ezd = const.tile([128, CHUNK], I8, tag=f"eztd{c}_{g}")
            rd_a = emit_exp(z[:, ts(0, CHUNK)], eza[:], False)
            rd_d = emit_exp(z[:, ts(1, CHUNK)], ezd[:], True)
            for s, (rb, rd) in enumerate(zip(rbs, [rd_a, rd_d])):
                nc.tensor.matmul(
                    q[0:OHW, :],
                    lhsT=ohsb[:, g, s],
                    rhs=rd,
                    start=(rb == 0),
                    stop=(rb == N_RB - 1),
                    skip_group_check=True,
                )

        def emit_groups(c, q, lo, hi):
            # First NSPLIT groups of chunk 0 and last NSPLIT groups of the
            # last chunk are split across BOTH exp engines (G=1 halves in
            # parallel): the exp pipeline ramps at double rate after the
            # boot DMA, and the end-of-kernel drain is half-group-deep.
            if c == 0 and lo == 0:
                for g in range(NSPLIT):
                    emit_split_group(c, q, g)
                lo = NSPLIT
            split_tail = c == N_CHUNKS - 1 and hi == N_GROUPS
            ngrp = hi - NSPLIT if split_tail else hi
            for g in range(lo, ngrp):
                rbs = [G * g + s for s in range(G)]
                z = zp.tile([128, G * CHUNK], F32, tag="z", name="z")
                for s, rb in enumerate(rbs):
                    nc.tensor.matmul(
                        z[:, ts(s, CHUNK)],
                        lhsT=w1(rb),
                        rhs=xtc_h[c],
                        start=True,
                        stop=True,
                        skip_group_check=True,
                    )
                dve = group_on_dve(c, g)
                ez = ezp.tile([128, G, CHUNK], I8 if dve else F8, tag="ez", name="ez")
                ez_rd = emit_exp(z[:], ez[:], dve)
                nc.tensor.matmul(
                    q[0:OHW, :],
                    lhsT=w2pair(g),
                    rhs=ez_rd,
                    start=(g == 0),
                    stop=(g == N_GROUPS - 1),
                    perf_mode=mybir.MatmulPerfMode.DoubleRow,
                    skip_group_check=True,
                )
            if split_tail:
                for g in range(N_GROUPS - NSPLIT, N_GROUPS):
                    emit_split_group(c, q, g)

        def emit_extract(c, q):
            # S_j = Q[1 + t_j, j]: mask away all but row 1+t_j, then a
            # ones-matmul reduces over partitions. Shipping only [2, 512]
            # keeps the end-of-kernel DMA tiny (a [101, 512] Q dump was
            # measured at 21 GB/s on a single DMA engine = 9.7us of tail).
            mk = mkp.tile([NCLS + 1, CHUNK], F16, tag="mk", name="mk")
            nc.vector.tensor_mul(mk[:], q[0 : NCLS + 1, :], cm[:, ts(c, CHUNK)])
            fs = fsp.tile([2, CHUNK], F32, tag="fs", name=f"fs{c}")
            stile = qp.tile([128, CHUNK], F32, tag="q", name=f"stile{c}")
            nc.tensor.matmul(
                stile[0:2, :],
                lhsT=sel[:],
                rhs=mk[:],
                start=True,
                stop=True,
                skip_group_check=True,
            )
            if c == N_CHUNKS - 1:
                nc.scalar.copy(fs[:], stile[0:2, :])
            else:
                nc.vector.tensor_copy(fs[:], stile[0:2, :])
            nc.sync.dma_start(fs_d[c][:], fs[:])

        q0 = qp.tile([128, CHUNK], F32, tag="q", name="q0")
        emit_groups(0, q0, 0, N_GROUPS)
        q1 = qp.tile([128, CHUNK], F32, tag="q", name="q1")
        # Chunk-0's extraction is emitted after chunk-1's pipeline is primed
        # so the extract matmul doesn't stall the PE FIFO at the transition.
        emit_groups(1, q1, 0, 4)
        emit_extract(0, q0)
        emit_groups(1, q1, 4, N_GROUPS)
        emit_extract(1, q1)

    _strip_self_engine_waits(nc)
    return nc


def _split_drain_waits(nc: bass.Bass, max_waits: int = 1) -> None:
    """walrus codegen caps sync waits per instruction (the kernel-tail drain
    waits on all 13 processors; a DMA whose round-robin semaphore is being
    recycled carries a recycle wait on top of its data wait). Hoist excess
    waits onto a chain of preceding drains on the same engine: engines run
    their streams in order, so waits satisfied by an earlier instruction
    cover the later one."""
    for bb in nc.main_func.blocks:
        out = []
        for ins in bb.instructions:
            si = ins.sync_info
            waits = list(si.on_wait) if si and si.on_wait else []
            if len(waits) > max_waits:
                chunks = [
                    waits[i : i + max_waits] for i in range(0, len(waits), max_waits)
                ]
                for j, ch in enumerate(chunks[:-1]):
                    out.append(
                        mybir.InstDrain(
                            name=f"{ins.name}-w{j}",
                            ins=[],
                            outs=[],
                            engine=ins.engine,
                            sync_info=mybir.SyncInfo(on_wait=ch, on_update=[]),
                        )
                    )
                ins.sync_info = mybir.SyncInfo(
                    on_wait=chunks[-1], on_update=list(si.on_update or [])
                )
            out.append(ins)
        bb.instructions[:] = out


def _strip_self_engine_waits(nc: bass.Bass) -> None:
    """Drop semaphore waits an engine instruction holds on its *own* engine's
    semaphore when it also waits on another engine (walrus rejects >1 sync
    wait on compute-engine instructions). Engines execute their instruction
    streams strictly in order, so a wait on the issuing engine's own
    semaphore is always satisfied by program order and removing it cannot
    reorder any access."""
    prefix = {
        mybir.EngineType.Activation: "Activation_",
        mybir.EngineType.PE: "PE_",
        mybir.EngineType.DVE: "DVE_",
        mybir.EngineType.Pool: "Pool_",
    }
    for bb in nc.main_func.blocks:
        for ins in bb.instructions:
            si = ins.sync_info
            if not si or not si.on_wait or len(si.on_wait) < 2:
                continue
            pref = prefix.get(ins.engine)
            if pref is None:
                continue
            kept = [w for w in si.on_wait if not (w.ant_name or "").startswith(pref)]
            if len(kept) != len(si.on_wait):
                ins.sync_info = mybir.SyncInfo(
                    on_wait=kept, on_update=list(si.on_update)
                )


def _get_program(split_waits: bool = True) -> bass.Bass:
    """split_waits rewrites the tail drain for walrus codegen (1 sync wait
    per instruction); CoreSim chokes on the synthetic drains, so the sim
    path requests the unsplit program."""
    global _PROGRAM, _PROGRAM_SPLIT
    if _PROGRAM is None:
        _PROGRAM = _build_program()
        _PROGRAM_SPLIT = False
    if split_waits and not _PROGRAM_SPLIT:
        _split_drain_waits(_PROGRAM)
        _PROGRAM_SPLIT = True
    return _PROGRAM


def _prepare_in_maps(out_1, out_2, target):
    f8 = mybir.dt.np(F8)
    x = np.concatenate(
        [np.asarray(out_1, np.float32), np.asarray(out_2, np.float32)], axis=0
    )
    xt = np.ascontiguousarray(x.astype(f8).T)  # [128, 8192] fp8e4
    t2 = np.concatenate([np.asarray(target), np.asarray(target)]).astype(np.int64)

    oh = np.zeros((TWO_B, OHW), f8)
    oh[:, 0] = 1.0  # ones column -> full_j row of Q (partition 0)
    oh[np.arange(TWO_B), 1 + t2] = 1.0
    # pack pairs for DoubleRow: [pair, k, p, c] -> [p, pair, k, c]
    ohpair = np.ascontiguousarray(
        oh.reshape(N_GROUPS, 2, 128, OHW).transpose(2, 0, 1, 3)
    )
    xt3 = xt.reshape(128, 8, 1024)
    w0 = np.ascontiguousarray(xt3[:, 0, 256:])
    wk = np.ascontiguousarray(xt3.transpose(1, 0, 2)[1:])

    in_maps = []
    for core in range(N_CORES):
        c0 = core * COLS_PER_CORE
        boot = np.ascontiguousarray(
            np.concatenate([xt[:, 0:256], xt[:, c0 : c0 + CHUNK]], axis=1)
        )
        tcols = t2[c0 : c0 + COLS_PER_CORE]
        cmask = (
            np.arange(NCLS + 1, dtype=np.int64)[:, None] == (1 + tcols)[None, :]
        ).astype(f8)
        cmask[0, :] = 1.0  # pass the Q ones-row (full) through the mask-mul
        sel = np.zeros((NCLS + 1, 2), np.float16)
        sel[0, 0] = 1.0        # col 0 picks mk[0] = full
        sel[1:, 1] = 1.0       # col 1 sums masked class rows = S
        in_maps.append(
            {
                "boot": boot,
                "w0": w0,
                "wk": wk,
                "oh8a": ohpair[:, 0:4],
                "oh8b": ohpair[:, 4:16],
                "oh8c": ohpair[:, 16:32],
                "cmask": cmask,
                "sel": sel,
                "xtc1": np.ascontiguousarray(xt[:, c0 + CHUNK : c0 + COLS_PER_CORE]),
            }
        )
    return in_maps


def _finish(q_per_core, target) -> np.ndarray:
    full = np.empty(TWO_B, np.float64)
    s = np.empty(TWO_B, np.float64)
    for core in range(N_CORES):
        qc = np.asarray(q_per_core[core], np.float64).reshape(N_CHUNKS, 2, CHUNK)
        c0 = core * COLS_PER_CORE
        for c in range(N_CHUNKS):
            cols = slice(c0 + c * CHUNK, c0 + (c + 1) * CHUNK)
            full[cols] = qc[c, 0, :]
            s[cols] = qc[c, 1, :]
    n = TWO_B - 2
    ng = full - s
    o1 = full - (1.0 - TAU_PLUS) * ng
    o2 = full + (n * TAU_PLUS - (1.0 - TAU_PLUS)) * ng
    loss = float(np.mean(np.log(o2) - np.log(o1)))
    return np.array(loss, dtype=np.float32)


def run(out_1, out_2, out_m, target, trace=False):
    """Run on hardware; returns (loss, exec_time_ns or None)."""
    nc = _get_program()
    in_maps = _prepare_in_maps(out_1, out_2, target)
    res = run_bass_kernel_spmd(nc, in_maps, list(range(N_CORES)), trace=trace)
    qs = [
        np.stack([res.results[i]["fs0"], res.results[i]["fs1"]])
        for i in range(N_CORES)
    ]
    return _finish(qs, target), res.exec_time_ns


def kernel(out_1, out_2, out_m, target):
    loss, _ = run(out_1, out_2, out_m, target, trace=False)
    return loss


# revision 27
# speedup vs baseline: 1.1892x; 1.1892x over previous
"""DebiasedPosLossV2 contrastive loss on 8 Trainium2 NeuronCores.

Math (reference, B=4096, D=128, TEMP=0.5, TAU=0.1):
    out = concat([out_1, out_2])            # [2B, D], rows L2-normalized
    sim = exp(out @ out.T / TEMP)           # [2B, 2B]
    full_i = sum_j sim_ij
    keep_ij = (j%B != i%B) & ~(t_i == t_j)  where t = concat([target, target])
    Ng_i = sum_j keep_ij * sim_ij
    loss = mean(-log(o1/o2)),  o1 = full - .9*Ng,  o2 = full + (n*.1-.9)*Ng

Key identity: t_j == t_i whenever j%B == i%B, so keep_ij == (t_i != t_j) and
    Ng_i = full_i - S_i,   S_i = sum_{j: t_j == t_i} sim_ij.

Sharding: every core holds the full X^T (all-gather done host-side by
replication) and owns a 1024-column strip of sim; column sums equal row sums
by symmetry. Per 512-column chunk, the core accumulates over all 64 row
blocks rb:
    Q[c, j] = sum_i [t_i == c] ez[i, j]   (one-hot matmul; row 0 = ones
                                           column -> full_j)
then extracts full_j = Q[0, j] and S_j = Q[1 + t_j, j] on-device (cmask
multiply + ones-matmul partition reduce) and ships only [full | S] = 4KB per
chunk; a [101, 512] Q dump was measured at 21 GB/s on a single DMA engine
(9.7us of tail), so the output must stay tiny. Host finishes with
o1/o2/log/mean in float64.

Engine budget per core: exp() over 8.4M elements would be 54.6us on ScalarE
alone (1 elem/cycle/lane @1.2GHz), and the two matmuls per 128x512 block
put TensorE at 55us (1 column/cycle @2.4GHz). Three measures get under
both limits:
  - exp is split across engines: even groups get true ScalarE exp into
    fp8e4m3; odd groups get a Schraudolph bit-trick exp on the otherwise
    idle VectorE: bits = int8(SCH8_A*z + SCH8_B) viewed as fp8, which is
    exp(2z) with ~+-5% ripple. The ripple is value-dependent but
    target-independent, so it applies the same multiplicative factor (in
    expectation) to full_j and S_j; o1/o2 are linear in (full, S), so the
    common factor cancels in o2/o1 and the final loss error is ~1e-5,
    far inside the 2e-2 budget.
  - everything is fp8e4m3: x entries (|x|<=1, unit rows) quantize to
    ~1.8% rel err, zero-mean, which washes out over 8192-term sums; input
    DMA bytes halve vs fp16.
  - the one-hot reduce runs as ONE DoubleRow fp8 matmul per row-block
    PAIR (stationary [128, 2, 128], moving [128, 2, 512], K=256): half
    the reduce instructions at 0.5 cycles/row, cutting TensorE to ~43us,
    which is the critical path.

Startup: DMA issues are spread across the sync/scalar/gpsimd queues (each
issue costs ~600ns serially per engine; walrus caps instructions at one
sync wait, so extra data/recycle waits are hoisted onto same-engine DRAIN
chains), and ~30 dummy N=128 matmuls on a memset tile warm the PE's HAM
clock gate (cold PE runs at 1.2GHz for the first ~3.4us of activity) while
the boot DMA is still in flight.
"""

import sys

if "/opt/trn_rl_repo" not in sys.path:
    sys.path.insert(0, "/opt/trn_rl_repo")

from contextlib import ExitStack

import numpy as np

import concourse.bass as bass
import concourse.mybir as mybir
import concourse.tile as tile
from concourse.bass import ds, ts
from concourse.bass_utils import run_bass_kernel_spmd

B = 4096
D = 128
TWO_B = 2 * B
TEMPERATURE = 0.5
TAU_PLUS = 0.1
N_CORES = 8
COLS_PER_CORE = TWO_B // N_CORES  # 1024
CHUNK = 512                       # psum bank width (fp32)
N_CHUNKS = COLS_PER_CORE // CHUNK  # 2
N_RB = TWO_B // 128               # 64 row blocks
G = 2                             # row blocks per group (z tile = 2 banks)
N_GROUPS = N_RB // G              # 32 groups per chunk
NCLS = 100                        # target values in [0, 100)
# one-hot layout: col 0 = ones (-> full row of Q), cols 1..100 = classes,
# cols 101..127 = zero pad (keeps PSUM APs partition-0 based and the
# 128-wide weight tile enables fast weight load).
OHW = 128
NWARM = 26                        # HAM warm-up matmuls (N=128 each)

# Schraudolph fp16 exp(2z): bits = int16(SCH_A*z + SCH_B) viewed as fp16.
# SCH_A = 1024 * 2/ln2; SCH_B = 15*1024 - 44.07 (centers the +-3% piecewise-
# linear ripple) + 0.5 (int conversion truncates).
SCH_A = 2954.639443
SCH_B = 15316.43
# Schraudolph fp8e4m3 exp(2z): bits = int8(SCH8_A*z + SCH8_B) viewed as fp8.
SCH8_A = 23.083120
SCH8_B = 56.1557

F16 = mybir.dt.float16
F32 = mybir.dt.float32
I16 = mybir.dt.int16
I8 = mybir.dt.int8
F8 = mybir.dt.float8e4

_PROGRAM = None
_PROGRAM_SPLIT = False


def group_on_dve(c: int, g: int) -> bool:
    """Which engine exponentiates group g of chunk c: False=ScalarE (true
    exp), True=VectorE (Schraudolph). Alternate for balanced load; chunk 1's
    group 5 goes to ScalarE so the DVE can absorb chunk-0's extract ops
    (mask-mul + stile copy, ~1.4us) without backing up the exp pipeline."""
    if c == 1 and g == 5:
        return False
    return g % 2 == 1


def _build_program() -> bass.Bass:
    nc = bass.Bass()

    # boot: everything group 0 needs in ONE descriptor:
    # [xt cols 0:256 | xtc chunk 0 | oh blocks 0,1] fp16 [128, 1024]
    boot_d = nc.declare_dram_parameter("boot", [128, 2 * 128 + CHUNK], F8, isOutput=False)
    # w0 = xt cols 256:1024
    w0_d = nc.declare_dram_parameter("w0", [128, 768], F8, isOutput=False)
    # wk[k] = xt cols (k+1)*1024:(k+2)*1024
    wk_d = nc.declare_dram_parameter("wk", [7, 128, 1024], F8, isOutput=False)
    # one-hot pair weights for the DoubleRow reduce: [p, pair, k, class]
    oh8a_d = nc.declare_dram_parameter("oh8a", [128, 4, 2, OHW], F8, isOutput=False)
    oh8b_d = nc.declare_dram_parameter("oh8b", [128, 12, 2, OHW], F8, isOutput=False)
    oh8c_d = nc.declare_dram_parameter("oh8c", [128, 16, 2, OHW], F8, isOutput=False)
    xtc1_d = nc.declare_dram_parameter("xtc1", [D, CHUNK], F8, isOutput=False)
    cm_d = nc.declare_dram_parameter("cmask", [NCLS + 1, COLS_PER_CORE], F8, isOutput=False)
    fs_d = [
        nc.declare_dram_parameter(f"fs{c}", [1, 2 * CHUNK], F32, isOutput=True)
        for c in range(N_CHUNKS)
    ]

    with ExitStack() as ctx:
        tc = ctx.enter_context(tile.TileContext(nc))
        const = ctx.enter_context(tc.tile_pool(name="const", bufs=1))
        ezp = ctx.enter_context(tc.tile_pool(name="ez", bufs=4))
        mkp = ctx.enter_context(tc.tile_pool(name="mk", bufs=2))
        fsp = ctx.enter_context(tc.tile_pool(name="fs", bufs=2))
        zp = ctx.enter_context(tc.tile_pool(name="z", bufs=3, space="PSUM"))
        qp = ctx.enter_context(tc.tile_pool(name="q", bufs=2, space="PSUM"))

        # --- SBUF tiles ---
        warm = const.tile([128, 128], F8, tag="warm")
        boot = const.tile([128, 2 * 128 + CHUNK], F8, tag="boot")
        w0 = const.tile([128, 768], F8, tag="w0")
        wks = [
            const.tile([128, 1024], F8, tag=f"wk{k}", name=f"wk{k}")
            for k in range(1, 8)
        ]
        ohsb = const.tile([128, N_GROUPS, 2, OHW], F8, tag="ohsb")
        xtc1 = const.tile([D, CHUNK], F8, tag="xtc1")

        # --- DMA issue schedule: spread across queues; each issue costs
        # ~600ns serially on its engine, and the boot transfer gates the
        # first real matmul, so boot goes first on sync while gpsimd memsets
        # the warm-up tile and vector fetches w0 in parallel. ---
        cm = const.tile([NCLS + 1, COLS_PER_CORE], F8, tag="cm")
        ones = const.tile([NCLS + 1, 1], F16, tag="ones")
        nc.gpsimd.memset(ones[:], 1.0)
        nc.gpsimd.memset(warm[:], 1.0)
        # gpsimd's SWDGE moves big transfers through one engine at ~26GB/s
        # (a 384KB input there stalled the PE 13us) -- inputs go ONLY on the
        # two hardware queues. sync's SP engine is otherwise idle, so it
        # carries the long list; scalar keeps <=3 so the auto-inserted
        # ACT_TABLE_LOAD (and first exp) isn't pushed past ~11us.
        nc.sync.dma_start(boot[:], boot_d[:])
        nc.scalar.dma_start(w0[:], w0_d[:])        # own queue: lands ~ with boot
        nc.sync.dma_start(ohsb[:, 0:4], oh8a_d[:])
        nc.scalar.dma_start(wks[0][:], wk_d[0])    # rb 8-15, needed ~13us
        nc.scalar.dma_start(ohsb[:, 4:16], oh8b_d[:])
        nc.sync.dma_start(wks[1][:], wk_d[1])
        nc.sync.dma_start(wks[2][:], wk_d[2])
        nc.sync.dma_start(wks[3][:], wk_d[3])
        nc.sync.dma_start(wks[4][:], wk_d[4])
        nc.sync.dma_start(wks[5][:], wk_d[5])
        nc.sync.dma_start(wks[6][:], wk_d[6])
        nc.sync.dma_start(ohsb[:, 16:32], oh8c_d[:])
        nc.sync.dma_start(cm[:], cm_d[:])
        nc.sync.dma_start(xtc1[:], xtc1_d[:])      # chunk 1, needed ~40us

        xtc_h = [boot[:, 256 : 256 + CHUNK], xtc1[:]]

        def w1(rb):  # lhsT for the z matmul of row block rb
            if rb < 2:
                return boot[:, ts(rb, 128)]
            if rb < 8:
                return w0[:, ts(rb - 2, 128)]
            return wks[rb // 8 - 1][:, ts(rb % 8, 128)]

        def w2pair(p):  # [128, 2, OHW] stationary pair for the DR reduce
            return ohsb[:, p]

        # --- PE HAM warm-up: ~30 junk matmuls (N=128) on the memset tile.
        # They burn the ~3.4us activity window while the boot DMA is in
        # flight so the real matmul stream starts at 2.4GHz. Output goes to
        # a z-pool tile that is recycled before the real groups need it. ---
        zw = zp.tile([128, G * CHUNK], F32, tag="z", name="zwarm")
        for _ in range(NWARM):
            nc.tensor.matmul(
                zw[:, 0:128],
                lhsT=warm[:],
                rhs=warm[:],
                start=True,
                stop=True,
                skip_group_check=True,
            )

        def emit_exp(z_slice, ez_ap, on_dve):
            """One exp pass over a z PSUM slice into an fp8 (or int8) SBUF
            tile AP; returns the fp8-typed AP the reduce matmul streams."""
            if on_dve:
                nc.vector.tensor_scalar(
                    ez_ap,
                    z_slice,
                    SCH8_A,
                    SCH8_B,
                    op0=mybir.AluOpType.mult,
                    op1=mybir.AluOpType.add,
                )
                return ez_ap.bitcast(F8)
            nc.scalar.activation(
                ez_ap,
                z_slice,
                mybir.ActivationFunctionType.Exp,
                scale=1.0 / TEMPERATURE,
            )
            return ez_ap

        NSPLIT = 2

        def emit_split_group(c, q, g):
            """One group as two G=1 halves, ScalarE + VectorE in parallel."""
            rbs = [G * g, G * g + 1]
            z = zp.tile([128, G * CHUNK], F32, tag="z", name="z")
            for s, rb in enumerate(rbs):
                nc.tensor.matmul(
                    z[:, ts(s, CHUNK)],
                    lhsT=w1(rb),
                    rhs=xtc_h[c],
                    start=True,
                    stop=True,
                    skip_group_check=True,
                )
            # dedicated tiles: pool reuse would add cross-engine WAW
            # waits; walrus caps compute instructions at one sync wait
            eza = const.tile([128, CHUNK], F8, tag=f"ezta{c}_{g}")
            ezd = const.tile([128, CHUNK], I8, tag=f"eztd{c}_{g}")
            rd_a = emit_exp(z[:, ts(0, CHUNK)], eza[:], False)
            rd_d = emit_exp(z[:, ts(1, CHUNK)], ezd[:], True)
            for s, (rb, rd) in enumerate(zip(rbs, [rd_a, rd_d])):
                nc.tensor.matmul(
                    q[0:OHW, :],
                    lhsT=ohsb[:, g, s],
                    rhs=rd,
                    start=(rb == 0),
                    stop=(rb == N_RB - 1),
                    skip_group_check=True,
                )

        def emit_groups(c, q, lo, hi):
            # First NSPLIT groups of chunk 0 and last NSPLIT groups of the
            # last chunk are split across BOTH exp engines (G=1 halves in
            # parallel): the exp pipeline ramps at double rate after the
            # boot DMA, and the end-of-kernel drain is half-group-deep.
            if c == 0 and lo == 0:
                for g in range(NSPLIT):
                    emit_split_group(c, q, g)
                lo = NSPLIT
            split_tail = c == N_CHUNKS - 1 and hi == N_GROUPS
            ngrp = hi - NSPLIT if split_tail else hi
            for g in range(lo, ngrp):
                rbs = [G * g + s for s in range(G)]
                z = zp.tile([128, G * CHUNK], F32, tag="z", name="z")
                for s, rb in enumerate(rbs):
                    nc.tensor.matmul(
                        z[:, ts(s, CHUNK)],
                        lhsT=w1(rb),
                        rhs=xtc_h[c],
                        start=True,
                        stop=True,
                        skip_group_check=True,
                    )
                dve = group_on_dve(c, g)
                ez = ezp.tile([128, G, CHUNK], I8 if dve else F8, tag="ez", name="ez")
                ez_rd = emit_exp(z[:], ez[:], dve)
                nc.tensor.matmul(
                    q[0:OHW, :],
                    lhsT=w2pair(g),
                    rhs=ez_rd,
                    start=(g == 0),
                    stop=(g == N_GROUPS - 1),
                    perf_mode=mybir.MatmulPerfMode.DoubleRow,
                    skip_group_check=True,
                )
            if split_tail:
                for g in range(N_GROUPS - NSPLIT, N_GROUPS):
                    emit_split_group(c, q, g)

        def emit_extract(c, q):
            # S_j = Q[1 + t_j, j]: mask away all but row 1+t_j, then a
            # ones-matmul reduces over partitions. Shipping only [2, 512]
            # keeps the end-of-kernel DMA tiny (a [101, 512] Q dump was
            # measured at 21 GB/s on a single DMA engine = 9.7us of tail).
            mk = mkp.tile([NCLS + 1, CHUNK], F16, tag="mk", name="mk")
            nc.vector.tensor_mul(mk[:], q[0 : NCLS + 1, :], cm[:, ts(c, CHUNK)])
            fs = fsp.tile([1, 2 * CHUNK], F32, tag="fs", name=f"fs{c}")
            nc.scalar.copy(fs[:, 0:CHUNK], q[0:1, :])
            stile = qp.tile([128, CHUNK], F32, tag="q", name=f"stile{c}")
            nc.tensor.matmul(
                stile[0:1, :],
                lhsT=ones[:],
                rhs=mk[:],
                start=True,
                stop=True,
                skip_group_check=True,
            )
            if c == N_CHUNKS - 1:
                nc.scalar.copy(fs[:, CHUNK:], stile[0:1, :])
            else:
                nc.vector.tensor_copy(fs[:, CHUNK:], stile[0:1, :])
            nc.sync.dma_start(fs_d[c][:], fs[:])

        q0 = qp.tile([128, CHUNK], F32, tag="q", name="q0")
        emit_groups(0, q0, 0, N_GROUPS)
        q1 = qp.tile([128, CHUNK], F32, tag="q", name="q1")
        # Chunk-0's extraction is emitted after chunk-1's pipeline is primed
        # so the extract matmul doesn't stall the PE FIFO at the transition.
        emit_groups(1, q1, 0, 4)
        emit_extract(0, q0)
        emit_groups(1, q1, 4, N_GROUPS)
        emit_extract(1, q1)

    _strip_self_engine_waits(nc)
    return nc


def _split_drain_waits(nc: bass.Bass, max_waits: int = 1) -> None:
    """walrus codegen caps sync waits per instruction (the kernel-tail drain
    waits on all 13 processors; a DMA whose round-robin semaphore is being
    recycled carries a recycle wait on top of its data wait). Hoist excess
    waits onto a chain of preceding drains on the same engine: engines run
    their streams in order, so waits satisfied by an earlier instruction
    cover the later one."""
    for bb in nc.main_func.blocks:
        out = []
        for ins in bb.instructions:
            si = ins.sync_info
            waits = list(si.on_wait) if si and si.on_wait else []
            if len(waits) > max_waits:
                chunks = [
                    waits[i : i + max_waits] for i in range(0, len(waits), max_waits)
                ]
                for j, ch in enumerate(chunks[:-1]):
                    out.append(
                        mybir.InstDrain(
                            name=f"{ins.name}-w{j}",
                            ins=[],
                            outs=[],
                            engine=ins.engine,
                            sync_info=mybir.SyncInfo(on_wait=ch, on_update=[]),
                        )
                    )
                ins.sync_info = mybir.SyncInfo(
                    on_wait=chunks[-1], on_update=list(si.on_update or [])
                )
            out.append(ins)
        bb.instructions[:] = out


def _strip_self_engine_waits(nc: bass.Bass) -> None:
    """Drop semaphore waits an engine instruction holds on its *own* engine's
    semaphore when it also waits on another engine (walrus rejects >1 sync
    wait on compute-engine instructions). Engines execute their instruction
    streams strictly in order, so a wait on the issuing engine's own
    semaphore is always satisfied by program order and removing it cannot
    reorder any access."""
    prefix = {
        mybir.EngineType.Activation: "Activation_",
        mybir.EngineType.PE: "PE_",
        mybir.EngineType.DVE: "DVE_",
        mybir.EngineType.Pool: "Pool_",
    }
    for bb in nc.main_func.blocks:
        for ins in bb.instructions:
            si = ins.sync_info
            if not si or not si.on_wait or len(si.on_wait) < 2:
                continue
            pref = prefix.get(ins.engine)
            if pref is None:
                continue
            kept = [w for w in si.on_wait if not (w.ant_name or "").startswith(pref)]
            if len(kept) != len(si.on_wait):
                ins.sync_info = mybir.SyncInfo(
                    on_wait=kept, on_update=list(si.on_update)
                )


def _get_program(split_waits: bool = True) -> bass.Bass:
    """split_waits rewrites the tail drain for walrus codegen (1 sync wait
    per instruction); CoreSim chokes on the synthetic drains, so the sim
    path requests the unsplit program."""
    global _PROGRAM, _PROGRAM_SPLIT
    if _PROGRAM is None:
        _PROGRAM = _build_program()
        _PROGRAM_SPLIT = False
    if split_waits and not _PROGRAM_SPLIT:
        _split_drain_waits(_PROGRAM)
        _PROGRAM_SPLIT = True
    return _PROGRAM


def _prepare_in_maps(out_1, out_2, target):
    f8 = mybir.dt.np(F8)
    x = np.concatenate(
        [np.asarray(out_1, np.float32), np.asarray(out_2, np.float32)], axis=0
    )
    xt = np.ascontiguousarray(x.astype(f8).T)  # [128, 8192] fp8e4
    t2 = np.concatenate([np.asarray(target), np.asarray(target)]).astype(np.int64)

    oh = np.zeros((TWO_B, OHW), f8)
    oh[:, 0] = 1.0  # ones column -> full_j row of Q (partition 0)
    oh[np.arange(TWO_B), 1 + t2] = 1.0
    # pack pairs for DoubleRow: [pair, k, p, c] -> [p, pair, k, c]
    ohpair = np.ascontiguousarray(
        oh.reshape(N_GROUPS, 2, 128, OHW).transpose(2, 0, 1, 3)
    )
    xt3 = xt.reshape(128, 8, 1024)
    w0 = np.ascontiguousarray(xt3[:, 0, 256:])
    wk = np.ascontiguousarray(xt3.transpose(1, 0, 2)[1:])

    in_maps = []
    for core in range(N_CORES):
        c0 = core * COLS_PER_CORE
        boot = np.ascontiguousarray(
            np.concatenate([xt[:, 0:256], xt[:, c0 : c0 + CHUNK]], axis=1)
        )
        tcols = t2[c0 : c0 + COLS_PER_CORE]
        cmask = (
            np.arange(NCLS + 1, dtype=np.int64)[:, None] == (1 + tcols)[None, :]
        ).astype(f8)
        in_maps.append(
            {
                "boot": boot,
                "w0": w0,
                "wk": wk,
                "oh8a": ohpair[:, 0:4],
                "oh8b": ohpair[:, 4:16],
                "oh8c": ohpair[:, 16:32],
                "cmask": cmask,
                "xtc1": np.ascontiguousarray(xt[:, c0 + CHUNK : c0 + COLS_PER_CORE]),
            }
        )
    return in_maps


def _finish(q_per_core, target) -> np.ndarray:
    full = np.empty(TWO_B, np.float64)
    s = np.empty(TWO_B, np.float64)
    for core in range(N_CORES):
        qc = np.asarray(q_per_core[core], np.float64).reshape(N_CHUNKS, 2, CHUNK)
        c0 = core * COLS_PER_CORE
        for c in range(N_CHUNKS):
            cols = slice(c0 + c * CHUNK, c0 + (c + 1) * CHUNK)
            full[cols] = qc[c, 0, :]
            s[cols] = qc[c, 1, :]
    n = TWO_B - 2
    ng = full - s
    o1 = full - (1.0 - TAU_PLUS) * ng
    o2 = full + (n * TAU_PLUS - (1.0 - TAU_PLUS)) * ng
    loss = float(np.mean(np.log(o2) - np.log(o1)))
    return np.array(loss, dtype=np.float32)


def run(out_1, out_2, out_m, target, trace=False):
    """Run on hardware; returns (loss, exec_time_ns or None)."""
    nc = _get_program()
    in_maps = _prepare_in_maps(out_1, out_2, target)
    res = run_bass_kernel_spmd(nc, in_maps, list(range(N_CORES)), trace=trace)
    qs = [
        np.stack([res.results[i]["fs0"], res.results[i]["fs1"]])
        for i in range(N_CORES)
    ]
    return _finish(qs, target), res.exec_time_ns


def kernel(out_1, out_2, out_m, target):
    loss, _ = run(out_1, out_2, out_m, target, trace=False)
    return loss


# revision 28
# speedup vs baseline: 1.1895x; 1.0002x over previous
"""DebiasedPosLossV2 contrastive loss on 8 Trainium2 NeuronCores.

Math (reference, B=4096, D=128, TEMP=0.5, TAU=0.1):
    out = concat([out_1, out_2])            # [2B, D], rows L2-normalized
    sim = exp(out @ out.T / TEMP)           # [2B, 2B]
    full_i = sum_j sim_ij
    keep_ij = (j%B != i%B) & ~(t_i == t_j)  where t = concat([target, target])
    Ng_i = sum_j keep_ij * sim_ij
    loss = mean(-log(o1/o2)),  o1 = full - .9*Ng,  o2 = full + (n*.1-.9)*Ng

Key identity: t_j == t_i whenever j%B == i%B, so keep_ij == (t_i != t_j) and
    Ng_i = full_i - S_i,   S_i = sum_{j: t_j == t_i} sim_ij.

Sharding: every core holds the full X^T (all-gather done host-side by
replication) and owns a 1024-column strip of sim; column sums equal row sums
by symmetry. Per 512-column chunk, the core accumulates over all 64 row
blocks rb:
    Q[c, j] = sum_i [t_i == c] ez[i, j]   (one-hot matmul; row 0 = ones
                                           column -> full_j)
then extracts full_j = Q[0, j] and S_j = Q[1 + t_j, j] on-device (cmask
multiply + ones-matmul partition reduce) and ships only [full | S] = 4KB per
chunk; a [101, 512] Q dump was measured at 21 GB/s on a single DMA engine
(9.7us of tail), so the output must stay tiny. Host finishes with
o1/o2/log/mean in float64.

Engine budget per core: exp() over 8.4M elements would be 54.6us on ScalarE
alone (1 elem/cycle/lane @1.2GHz), and the two matmuls per 128x512 block
put TensorE at 55us (1 column/cycle @2.4GHz). Three measures get under
both limits:
  - exp is split across engines: even groups get true ScalarE exp into
    fp8e4m3; odd groups get a Schraudolph bit-trick exp on the otherwise
    idle VectorE: bits = int8(SCH8_A*z + SCH8_B) viewed as fp8, which is
    exp(2z) with ~+-5% ripple. The ripple is value-dependent but
    target-independent, so it applies the same multiplicative factor (in
    expectation) to full_j and S_j; o1/o2 are linear in (full, S), so the
    common factor cancels in o2/o1 and the final loss error is ~1e-5,
    far inside the 2e-2 budget.
  - everything is fp8e4m3: x entries (|x|<=1, unit rows) quantize to
    ~1.8% rel err, zero-mean, which washes out over 8192-term sums; input
    DMA bytes halve vs fp16.
  - the one-hot reduce runs as ONE DoubleRow fp8 matmul per row-block
    PAIR (stationary [128, 2, 128], moving [128, 2, 512], K=256): half
    the reduce instructions at 0.5 cycles/row, cutting TensorE to ~43us,
    which is the critical path.

Startup: DMA issues are spread across the sync/scalar/gpsimd queues (each
issue costs ~600ns serially per engine; walrus caps instructions at one
sync wait, so extra data/recycle waits are hoisted onto same-engine DRAIN
chains), and ~30 dummy N=128 matmuls on a memset tile warm the PE's HAM
clock gate (cold PE runs at 1.2GHz for the first ~3.4us of activity) while
the boot DMA is still in flight.
"""

import sys

if "/opt/trn_rl_repo" not in sys.path:
    sys.path.insert(0, "/opt/trn_rl_repo")

from contextlib import ExitStack

import numpy as np

import concourse.bass as bass
import concourse.mybir as mybir
import concourse.tile as tile
from concourse.bass import ds, ts
from concourse.bass_utils import run_bass_kernel_spmd

B = 4096
D = 128
TWO_B = 2 * B
TEMPERATURE = 0.5
TAU_PLUS = 0.1
N_CORES = 8
COLS_PER_CORE = TWO_B // N_CORES  # 1024
CHUNK = 512                       # psum bank width (fp32)
N_CHUNKS = COLS_PER_CORE // CHUNK  # 2
N_RB = TWO_B // 128               # 64 row blocks
G = 2                             # row blocks per group (z tile = 2 banks)
N_GROUPS = N_RB // G              # 32 groups per chunk
NCLS = 100                        # target values in [0, 100)
# one-hot layout: col 0 = ones (-> full row of Q), cols 1..100 = classes,
# cols 101..127 = zero pad (keeps PSUM APs partition-0 based and the
# 128-wide weight tile enables fast weight load).
OHW = 128
NWARM = 26                        # HAM warm-up matmuls (N=128 each)

# Schraudolph fp16 exp(2z): bits = int16(SCH_A*z + SCH_B) viewed as fp16.
# SCH_A = 1024 * 2/ln2; SCH_B = 15*1024 - 44.07 (centers the +-3% piecewise-
# linear ripple) + 0.5 (int conversion truncates).
SCH_A = 2954.639443
SCH_B = 15316.43
# Schraudolph fp8e4m3 exp(2z): bits = int8(SCH8_A*z + SCH8_B) viewed as fp8.
SCH8_A = 23.083120
SCH8_B = 56.1557

F16 = mybir.dt.float16
F32 = mybir.dt.float32
I16 = mybir.dt.int16
I8 = mybir.dt.int8
F8 = mybir.dt.float8e4

_PROGRAM = None
_PROGRAM_SPLIT = False


def group_on_dve(c: int, g: int) -> bool:
    """Which engine exponentiates group g of chunk c: False=ScalarE (true
    exp), True=VectorE (Schraudolph). Alternate for balanced load; chunk 1's
    group 5 goes to ScalarE so the DVE can absorb chunk-0's extract ops
    (mask-mul + stile copy, ~1.4us) without backing up the exp pipeline."""
    if c == 1 and g == 5:
        return False
    return g % 2 == 1


def _build_program() -> bass.Bass:
    nc = bass.Bass()

    # boot: everything group 0 needs in ONE descriptor:
    # [xt cols 0:256 | xtc chunk 0 | oh blocks 0,1] fp16 [128, 1024]
    boot_d = nc.declare_dram_parameter("boot", [128, 2 * 128 + CHUNK], F8, isOutput=False)
    # w0 = xt cols 256:1024
    w0_d = nc.declare_dram_parameter("w0", [128, 768], F8, isOutput=False)
    # wk[k] = xt cols (k+1)*1024:(k+2)*1024
    wk_d = nc.declare_dram_parameter("wk", [7, 128, 1024], F8, isOutput=False)
    # one-hot pair weights for the DoubleRow reduce: [p, pair, k, class]
    oh8a_d = nc.declare_dram_parameter("oh8a", [128, 4, 2, OHW], F8, isOutput=False)
    oh8b_d = nc.declare_dram_parameter("oh8b", [128, 12, 2, OHW], F8, isOutput=False)
    oh8c_d = nc.declare_dram_parameter("oh8c", [128, 16, 2, OHW], F8, isOutput=False)
    xtc1_d = nc.declare_dram_parameter("xtc1", [D, CHUNK], F8, isOutput=False)
    cm_d = nc.declare_dram_parameter("cmask", [NCLS + 1, COLS_PER_CORE], F8, isOutput=False)
    fs_d = [
        nc.declare_dram_parameter(f"fs{c}", [1, 2 * CHUNK], F32, isOutput=True)
        for c in range(N_CHUNKS)
    ]

    with ExitStack() as ctx:
        tc = ctx.enter_context(tile.TileContext(nc))
        const = ctx.enter_context(tc.tile_pool(name="const", bufs=1))
        ezp = ctx.enter_context(tc.tile_pool(name="ez", bufs=4))
        mkp = ctx.enter_context(tc.tile_pool(name="mk", bufs=2))
        fsp = ctx.enter_context(tc.tile_pool(name="fs", bufs=2))
        zp = ctx.enter_context(tc.tile_pool(name="z", bufs=3, space="PSUM"))
        qp = ctx.enter_context(tc.tile_pool(name="q", bufs=2, space="PSUM"))

        # --- SBUF tiles ---
        warm = const.tile([128, 128], F8, tag="warm")
        boot = const.tile([128, 2 * 128 + CHUNK], F8, tag="boot")
        w0 = const.tile([128, 768], F8, tag="w0")
        wks = [
            const.tile([128, 1024], F8, tag=f"wk{k}", name=f"wk{k}")
            for k in range(1, 8)
        ]
        ohsb = const.tile([128, N_GROUPS, 2, OHW], F8, tag="ohsb")
        xtc1 = const.tile([D, CHUNK], F8, tag="xtc1")

        # --- DMA issue schedule: spread across queues; each issue costs
        # ~600ns serially on its engine, and the boot transfer gates the
        # first real matmul, so boot goes first on sync while gpsimd memsets
        # the warm-up tile and vector fetches w0 in parallel. ---
        cm = const.tile([NCLS + 1, COLS_PER_CORE], F8, tag="cm")
        ones = const.tile([NCLS + 1, 1], F16, tag="ones")
        nc.gpsimd.memset(ones[:], 1.0)
        nc.gpsimd.memset(warm[:], 1.0)
        # gpsimd's SWDGE moves big transfers through one engine at ~26GB/s
        # (a 384KB input there stalled the PE 13us) -- inputs go ONLY on the
        # two hardware queues. sync's SP engine is otherwise idle, so it
        # carries the long list; scalar keeps <=3 so the auto-inserted
        # ACT_TABLE_LOAD (and first exp) isn't pushed past ~11us.
        nc.sync.dma_start(boot[:], boot_d[:])
        nc.scalar.dma_start(w0[:], w0_d[:])        # own queue: lands ~ with boot
        nc.sync.dma_start(ohsb[:, 0:4], oh8a_d[:])
        nc.scalar.dma_start(wks[0][:], wk_d[0])    # rb 8-15, needed ~13us
        nc.scalar.dma_start(ohsb[:, 4:16], oh8b_d[:])
        nc.sync.dma_start(wks[1][:], wk_d[1])
        nc.sync.dma_start(wks[2][:], wk_d[2])
        nc.sync.dma_start(wks[3][:], wk_d[3])
        nc.sync.dma_start(wks[4][:], wk_d[4])
        nc.sync.dma_start(wks[5][:], wk_d[5])
        nc.sync.dma_start(wks[6][:], wk_d[6])
        nc.sync.dma_start(ohsb[:, 16:32], oh8c_d[:])
        nc.sync.dma_start(cm[:], cm_d[:])
        nc.sync.dma_start(xtc1[:], xtc1_d[:])      # chunk 1, needed ~40us

        xtc_h = [boot[:, 256 : 256 + CHUNK], xtc1[:]]

        def w1(rb):  # lhsT for the z matmul of row block rb
            if rb < 2:
                return boot[:, ts(rb, 128)]
            if rb < 8:
                return w0[:, ts(rb - 2, 128)]
            return wks[rb // 8 - 1][:, ts(rb % 8, 128)]

        def w2pair(p):  # [128, 2, OHW] stationary pair for the DR reduce
            return ohsb[:, p]

        # --- PE HAM warm-up: ~30 junk matmuls (N=128) on the memset tile.
        # They burn the ~3.4us activity window while the boot DMA is in
        # flight so the real matmul stream starts at 2.4GHz. Output goes to
        # a z-pool tile that is recycled before the real groups need it. ---
        zw = zp.tile([128, G * CHUNK], F32, tag="z", name="zwarm")
        for _ in range(NWARM):
            nc.tensor.matmul(
                zw[:, 0:128],
                lhsT=warm[:],
                rhs=warm[:],
                start=True,
                stop=True,
                skip_group_check=True,
            )

        def emit_exp(z_slice, ez_ap, on_dve):
            """One exp pass over a z PSUM slice into an fp8 (or int8) SBUF
            tile AP; returns the fp8-typed AP the reduce matmul streams."""
            if on_dve:
                nc.vector.tensor_scalar(
                    ez_ap,
                    z_slice,
                    SCH8_A,
                    SCH8_B,
                    op0=mybir.AluOpType.mult,
                    op1=mybir.AluOpType.add,
                )
                return ez_ap.bitcast(F8)
            nc.scalar.activation(
                ez_ap,
                z_slice,
                mybir.ActivationFunctionType.Exp,
                scale=1.0 / TEMPERATURE,
            )
            return ez_ap

        NSPLIT = 2

        def emit_split_group(c, q, g):
            """One group as two G=1 halves, ScalarE + VectorE in parallel."""
            rbs = [G * g, G * g + 1]
            z = zp.tile([128, G * CHUNK], F32, tag="z", name="z")
            for s, rb in enumerate(rbs):
                nc.tensor.matmul(
                    z[:, ts(s, CHUNK)],
                    lhsT=w1(rb),
                    rhs=xtc_h[c],
                    start=True,
                    stop=True,
                    skip_group_check=True,
                )
            # dedicated tiles: pool reuse would add cross-engine WAW
            # waits; walrus caps compute instructions at one sync wait
            eza = const.tile([128, CHUNK], F8, tag=f"ezta{c}_{g}")
            ezd = const.tile([128, CHUNK], I8, tag=f"eztd{c}_{g}")
            rd_a = emit_exp(z[:, ts(0, CHUNK)], eza[:], False)
            rd_d = emit_exp(z[:, ts(1, CHUNK)], ezd[:], True)
            for s, (rb, rd) in enumerate(zip(rbs, [rd_a, rd_d])):
                nc.tensor.matmul(
                    q[0:OHW, :],
                    lhsT=ohsb[:, g, s],
                    rhs=rd,
                    start=(rb == 0),
                    stop=(rb == N_RB - 1),
                    skip_group_check=True,
                )

        def emit_groups(c, q, lo, hi):
            # First NSPLIT groups of chunk 0 and last NSPLIT groups of the
            # last chunk are split across BOTH exp engines (G=1 halves in
            # parallel): the exp pipeline ramps at double rate after the
            # boot DMA, and the end-of-kernel drain is half-group-deep.
            if c == 0 and lo == 0:
                for g in range(NSPLIT):
                    emit_split_group(c, q, g)
                lo = NSPLIT
            split_tail = c == N_CHUNKS - 1 and hi == N_GROUPS
            ngrp = hi - NSPLIT if split_tail else hi
            for g in range(lo, ngrp):
                rbs = [G * g + s for s in range(G)]
                z = zp.tile([128, G * CHUNK], F32, tag="z", name="z")
                for s, rb in enumerate(rbs):
                    nc.tensor.matmul(
                        z[:, ts(s, CHUNK)],
                        lhsT=w1(rb),
                        rhs=xtc_h[c],
                        start=True,
                        stop=True,
                        skip_group_check=True,
                    )
                dve = group_on_dve(c, g)
                ez = ezp.tile([128, G, CHUNK], I8 if dve else F8, tag="ez", name="ez")
                ez_rd = emit_exp(z[:], ez[:], dve)
                nc.tensor.matmul(
                    q[0:OHW, :],
                    lhsT=w2pair(g),
                    rhs=ez_rd,
                    start=(g == 0),
                    stop=(g == N_GROUPS - 1),
                    perf_mode=mybir.MatmulPerfMode.DoubleRow,
                    skip_group_check=True,
                )
            if split_tail:
                for g in range(N_GROUPS - NSPLIT, N_GROUPS):
                    emit_split_group(c, q, g)

        def emit_extract(c, q):
            # S_j = Q[1 + t_j, j]: mask away all but row 1+t_j, then a
            # ones-matmul reduces over partitions. Shipping only [2, 512]
            # keeps the end-of-kernel DMA tiny (a [101, 512] Q dump was
            # measured at 21 GB/s on a single DMA engine = 9.7us of tail).
            mk = mkp.tile([NCLS + 1, CHUNK], F16, tag="mk", name="mk")
            nc.vector.tensor_mul(mk[:], q[0 : NCLS + 1, :], cm[:, ts(c, CHUNK)])
            fs = fsp.tile([1, 2 * CHUNK], F32, tag="fs", name=f"fs{c}")
            nc.scalar.copy(fs[:, 0:CHUNK], q[0:1, :])
            stile = qp.tile([128, CHUNK], F32, tag="q", name=f"stile{c}")
            nc.tensor.matmul(
                stile[0:1, :],
                lhsT=ones[:],
                rhs=mk[:],
                start=True,
                stop=True,
                skip_group_check=True,
            )
            if c == N_CHUNKS - 1:
                nc.scalar.copy(fs[:, CHUNK:], stile[0:1, :])
            else:
                nc.vector.tensor_copy(fs[:, CHUNK:], stile[0:1, :])
            nc.sync.dma_start(fs_d[c][:], fs[:])

        q0 = qp.tile([128, CHUNK], F32, tag="q", name="q0")
        emit_groups(0, q0, 0, N_GROUPS)
        q1 = qp.tile([128, CHUNK], F32, tag="q", name="q1")
        emit_groups(1, q1, 0, N_GROUPS)
        # BOTH extracts run in the kernel drain: the exp engines have only
        # ~110ns slack per group, so extract ops injected mid-stream cost a
        # multi-us recovery limp; at the drain DVE/ScalarE are idle. q0's
        # PSUM bank simply stays live until here (2 q banks + 6 z banks).
        emit_extract(0, q0)
        emit_extract(1, q1)

    _strip_self_engine_waits(nc)
    return nc


def _split_drain_waits(nc: bass.Bass, max_waits: int = 1) -> None:
    """walrus codegen caps sync waits per instruction (the kernel-tail drain
    waits on all 13 processors; a DMA whose round-robin semaphore is being
    recycled carries a recycle wait on top of its data wait). Hoist excess
    waits onto a chain of preceding drains on the same engine: engines run
    their streams in order, so waits satisfied by an earlier instruction
    cover the later one."""
    for bb in nc.main_func.blocks:
        out = []
        for ins in bb.instructions:
            si = ins.sync_info
            waits = list(si.on_wait) if si and si.on_wait else []
            if len(waits) > max_waits:
                chunks = [
                    waits[i : i + max_waits] for i in range(0, len(waits), max_waits)
                ]
                for j, ch in enumerate(chunks[:-1]):
                    out.append(
                        mybir.InstDrain(
                            name=f"{ins.name}-w{j}",
                            ins=[],
                            outs=[],
                            engine=ins.engine,
                            sync_info=mybir.SyncInfo(on_wait=ch, on_update=[]),
                        )
                    )
                ins.sync_info = mybir.SyncInfo(
                    on_wait=chunks[-1], on_update=list(si.on_update or [])
                )
            out.append(ins)
        bb.instructions[:] = out


def _strip_self_engine_waits(nc: bass.Bass) -> None:
    """Drop semaphore waits an engine instruction holds on its *own* engine's
    semaphore when it also waits on another engine (walrus rejects >1 sync
    wait on compute-engine instructions). Engines execute their instruction
    streams strictly in order, so a wait on the issuing engine's own
    semaphore is always satisfied by program order and removing it cannot
    reorder any access."""
    prefix = {
        mybir.EngineType.Activation: "Activation_",
        mybir.EngineType.PE: "PE_",
        mybir.EngineType.DVE: "DVE_",
        mybir.EngineType.Pool: "Pool_",
    }
    for bb in nc.main_func.blocks:
        for ins in bb.instructions:
            si = ins.sync_info
            if not si or not si.on_wait or len(si.on_wait) < 2:
                continue
            pref = prefix.get(ins.engine)
            if pref is None:
                continue
            kept = [w for w in si.on_wait if not (w.ant_name or "").startswith(pref)]
            if len(kept) != len(si.on_wait):
                ins.sync_info = mybir.SyncInfo(
                    on_wait=kept, on_update=list(si.on_update)
                )


def _get_program(split_waits: bool = True) -> bass.Bass:
    """split_waits rewrites the tail drain for walrus codegen (1 sync wait
    per instruction); CoreSim chokes on the synthetic drains, so the sim
    path requests the unsplit program."""
    global _PROGRAM, _PROGRAM_SPLIT
    if _PROGRAM is None:
        _PROGRAM = _build_program()
        _PROGRAM_SPLIT = False
    if split_waits and not _PROGRAM_SPLIT:
        _split_drain_waits(_PROGRAM)
        _PROGRAM_SPLIT = True
    return _PROGRAM


def _prepare_in_maps(out_1, out_2, target):
    f8 = mybir.dt.np(F8)
    x = np.concatenate(
        [np.asarray(out_1, np.float32), np.asarray(out_2, np.float32)], axis=0
    )
    xt = np.ascontiguousarray(x.astype(f8).T)  # [128, 8192] fp8e4
    t2 = np.concatenate([np.asarray(target), np.asarray(target)]).astype(np.int64)

    oh = np.zeros((TWO_B, OHW), f8)
    oh[:, 0] = 1.0  # ones column -> full_j row of Q (partition 0)
    oh[np.arange(TWO_B), 1 + t2] = 1.0
    # pack pairs for DoubleRow: [pair, k, p, c] -> [p, pair, k, c]
    ohpair = np.ascontiguousarray(
        oh.reshape(N_GROUPS, 2, 128, OHW).transpose(2, 0, 1, 3)
    )
    xt3 = xt.reshape(128, 8, 1024)
    w0 = np.ascontiguousarray(xt3[:, 0, 256:])
    wk = np.ascontiguousarray(xt3.transpose(1, 0, 2)[1:])

    in_maps = []
    for core in range(N_CORES):
        c0 = core * COLS_PER_CORE
        boot = np.ascontiguousarray(
            np.concatenate([xt[:, 0:256], xt[:, c0 : c0 + CHUNK]], axis=1)
        )
        tcols = t2[c0 : c0 + COLS_PER_CORE]
        cmask = (
            np.arange(NCLS + 1, dtype=np.int64)[:, None] == (1 + tcols)[None, :]
        ).astype(f8)
        in_maps.append(
            {
                "boot": boot,
                "w0": w0,
                "wk": wk,
                "oh8a": ohpair[:, 0:4],
                "oh8b": ohpair[:, 4:16],
                "oh8c": ohpair[:, 16:32],
                "cmask": cmask,
                "xtc1": np.ascontiguousarray(xt[:, c0 + CHUNK : c0 + COLS_PER_CORE]),
            }
        )
    return in_maps


def _finish(q_per_core, target) -> np.ndarray:
    full = np.empty(TWO_B, np.float64)
    s = np.empty(TWO_B, np.float64)
    for core in range(N_CORES):
        qc = np.asarray(q_per_core[core], np.float64).reshape(N_CHUNKS, 2, CHUNK)
        c0 = core * COLS_PER_CORE
        for c in range(N_CHUNKS):
            cols = slice(c0 + c * CHUNK, c0 + (c + 1) * CHUNK)
            full[cols] = qc[c, 0, :]
            s[cols] = qc[c, 1, :]
    n = TWO_B - 2
    ng = full - s
    o1 = full - (1.0 - TAU_PLUS) * ng
    o2 = full + (n * TAU_PLUS - (1.0 - TAU_PLUS)) * ng
    loss = float(np.mean(np.log(o2) - np.log(o1)))
    return np.array(loss, dtype=np.float32)


def run(out_1, out_2, out_m, target, trace=False):
    """Run on hardware; returns (loss, exec_time_ns or None)."""
    nc = _get_program()
    in_maps = _prepare_in_maps(out_1, out_2, target)
    res = run_bass_kernel_spmd(nc, in_maps, list(range(N_CORES)), trace=trace)
    qs = [
        np.stack([res.results[i]["fs0"], res.results[i]["fs1"]])
        for i in range(N_CORES)
    ]
    return _finish(qs, target), res.exec_time_ns


def kernel(out_1, out_2, out_m, target):
    loss, _ = run(out_1, out_2, out_m, target, trace=False)
    return loss
